# revision 1
# baseline (speedup 1.0000x reference)
"""Multi-head attention (B=2, S=2048, D=1024, H=16) on 8 Trainium2 NeuronCores.

Sharding: core i handles batch b = i//4 and head-group hg = i%4 (4 heads,
256 channels).  Per-head Q/K projection weights are replicated; the fc layer
is sharded over its contraction dim (each core contributes a partial y that
the host sums per batch).  Wv is folded into Wfc on the host (exact, since
softmax rows sum to 1 the bv contribution folds into bfc).

Device algorithm per core (all matmuls f32r, psum f32):
  - heads processed as PAIRS: head A on SBUF partitions 0-63, head B on
    64-127, so the K=64 score matmuls run as concurrent 64-row PE tiles
  - q'T/k'T projections: head B via a row-tiled M=128 matmul with
    block-diagonal weights, head A plain M=64 (scale 1/sqrt(64) folded in Wq)
  - scores computed transposed: S_t[k,q] = k'T_slice.T @ q'T (k on
    partitions); exp on ScalarE (no max-subtraction; |scores| <~ 2 so exp
    is safe), one N=1024 activation per k-tile covering both heads
  - AV: lhsT = [V_h | ones] (65 cols) so row 64 of the psum accumulator is
    the softmax denominator; accumulators are evacuated to SBUF and
    normalization is deferred one round (reciprocal -> ones-matmul
    partition-broadcast -> multiply); head B's normalized oT additionally
    goes through an identity-shift matmul to partitions 64-127
  - fc: y[s,c] accumulated over the two head-pairs with K=128 matmuls,
    emitted as single-matmul closures interleaved one-per-k-tile into the
    next q-window's attention so they hide under the ScalarE exp stream
"""


import sys

import numpy as np

if "/opt/trn_rl_repo" not in sys.path:
    sys.path.insert(0, "/opt/trn_rl_repo")

HEAD = 16
B, S, D = 2, 2048, 1024
HD = 64
HPC = 4          # heads per core
CH = HPC * HD    # channels per core
N_CORES = 8

_CACHE = {}
LAST_RESULTS = None


def _build():
    import concourse.tile as tile
    from concourse import bacc, mybir

    f32 = mybir.dt.float32
    f32r = mybir.dt.float32r
    EXP = mybir.ActivationFunctionType.Exp

    nc = bacc.Bacc("TRN2", target_bir_lowering=False, debug=False,
                   num_devices=N_CORES)

    # unused internal tensor whose name varies per retry: changes the BIR
    # content hash so a retry never reuses a possibly-corrupt cached NEFF
    nonce = _CACHE.get("nonce", 0)
    if nonce:
        nc.dram_tensor(f"retry_nonce_{nonce}", [1, 1], mybir.dt.float32)

    qt_d = nc.dram_tensor("qt", [CH, S], f32r, kind="ExternalInput")
    kt_d = nc.dram_tensor("kt", [CH, S], f32r, kind="ExternalInput")
    v1_d = nc.dram_tensor("v1", [S, 65 * HPC], f32r, kind="ExternalInput")
    wq_d = nc.dram_tensor("wqt", [2 * HD, 2 * HD], f32r, kind="ExternalInput")
    wk_d = nc.dram_tensor("wkt", [2 * HD, 2 * HD], f32r, kind="ExternalInput")
    bq_d = nc.dram_tensor("bq", [2 * HD, 1], f32, kind="ExternalInput")
    bk_d = nc.dram_tensor("bk", [2 * HD, 1], f32, kind="ExternalInput")
    wf_d = nc.dram_tensor("wfct", [CH, D], f32r, kind="ExternalInput")
    on_d = nc.dram_tensor("ones", [1, HD], f32r, kind="ExternalInput")
    ish_d = nc.dram_tensor("ishift", [HD, 2 * HD], f32r, kind="ExternalInput")
    y_d = nc.dram_tensor("y", [S, D], f32, kind="ExternalOutput")

    with tile.TileContext(nc) as tc, nc.allow_low_precision(
            reason="f32r tiles feed tensor-engine matmuls; psum stays f32"):
        with (
            tc.tile_pool(name="consts", bufs=1) as consts,
            tc.tile_pool(name="vpool", bufs=1) as vpool,
            tc.tile_pool(name="qk_in", bufs=2) as qk_in,
            tc.tile_pool(name="qk_proj", bufs=2) as qk_proj,
            tc.tile_pool(name="ot", bufs=1) as otp,
            tc.tile_pool(name="exp", bufs=3) as expp,
            tc.tile_pool(name="small", bufs=2) as small,
            tc.tile_pool(name="ysb", bufs=3) as ysb,
            tc.tile_pool(name="ps_score", bufs=2, space="PSUM") as ps_score,
            tc.tile_pool(name="ps_av", bufs=2, space="PSUM") as ps_av,
            tc.tile_pool(name="ps_misc", bufs=1, space="PSUM") as ps_misc,
        ):
            # ---------------- constants ----------------
            # only wk/wq gate the first projection; the rest can trail the
            # first input chunks
            wq_s = consts.tile([2 * HD, 2 * HD], f32r, tag="wq")
            wk_s = consts.tile([2 * HD, 2 * HD], f32r, tag="wk")
            bq_s = consts.tile([2 * HD, 1], f32, tag="bq")
            bk_s = consts.tile([2 * HD, 1], f32, tag="bk")
            ones_s = consts.tile([65, HD], f32r, tag="ones")
            ish_s = consts.tile([HD, 2 * HD], f32r, tag="ishift")

            def emit_late_consts():
                nc.sync.dma_start(out=bk_s, in_=bk_d[:, :])
                nc.sync.dma_start(out=bq_s, in_=bq_d[:, :])
                nc.sync.dma_start(out=ones_s[64:65, :], in_=on_d[:, :])
                nc.sync.dma_start(out=ish_s, in_=ish_d[:, :])
            # ---------------- projections ----------------
            # chunked input DMAs so the first proj matmul starts early;
            # j=0 inputs first, then v1 (needed from the first av), then the
            # remaining heads, then wfct (needed only by fc, much later)
            qp_s, kp_s = [], []
            v1_s = []
            wf_s = []
            deferred_qt = []

            def emit_proj(p):
                # head pair p: head 2p on partitions 0-63, head 2p+1 on
                # partitions 64-127 (concurrent 64x64 PE tiles T0 / T10)
                qt_t = qk_in.tile([2 * HD, S], f32r, tag="qt_in",
                                  name=f"qt_in{p}")
                kt_t = qk_in.tile([2 * HD, S], f32r, tag="kt_in",
                                  name=f"kt_in{p}")
                if p == 0:
                    # critical-path order: the first k-proj matmul needs only
                    # wk + kt chunk 0; scores consume kp chunk-by-chunk, but
                    # qt chunks 1-3 are not needed until the second q-window,
                    # so defer them until after the v1 loads
                    nc.sync.dma_start(out=wk_s, in_=wk_d[:, :])
                    nc.sync.dma_start(out=kt_t[:, 0:512],
                                      in_=kt_d[0:128, 0:512])
                    nc.sync.dma_start(out=wq_s, in_=wq_d[:, :])
                    nc.sync.dma_start(out=qt_t[:, 0:512],
                                      in_=qt_d[0:128, 0:512])
                    emit_late_consts()
                    for c in range(1, S // 512):
                        sl = slice(512 * c, 512 * c + 512)
                        nc.sync.dma_start(out=kt_t[:, sl],
                                          in_=kt_d[0:128, sl])
                    for c in range(1, S // 512):
                        sl = slice(512 * c, 512 * c + 512)
                        nc.sync.dma_start(out=qt_t[:, sl],
                                          in_=qt_d[0:128, sl])
                else:
                    for c in range(S // 512):
                        sl = slice(512 * c, 512 * c + 512)
                        nc.sync.dma_start(out=kt_t[:, sl],
                                          in_=kt_d[128 * p:128 * p + 128, sl])
                        nc.sync.dma_start(out=qt_t[:, sl],
                                          in_=qt_d[128 * p:128 * p + 128, sl])
                qp = qk_proj.tile([2 * HD, S], f32r, tag="qp", name=f"qp{p}")
                kp = qk_proj.tile([2 * HD, S], f32r, tag="kp", name=f"kp{p}")
                for qb in range(S // 512):
                    sl = slice(512 * qb, 512 * qb + 512)
                    # head B: row-tiled M=128 matmul with block-diag weights
                    # (only rows 64-127 valid); head A: plain M=64 matmul in
                    # a separate psum slot (same-bank double-write is a HW
                    # runtime error)
                    pk1 = ps_misc.tile([128, 512], f32, tag="misc",
                                       name=f"pk1{p}_{qb}")
                    nc.tensor.matmul(pk1, wk_s[64:128, :],
                                     kt_t[64:128, sl], start=True, stop=True)
                    nc.vector.tensor_scalar_add(kp[64:128, sl],
                                                pk1[64:128, :], bk_s[64:128])
                    pk2 = ps_misc.tile([128, 512], f32, tag="misc",
                                       name=f"pk2{p}_{qb}")
                    nc.tensor.matmul(pk2[0:64, :], wk_s[0:64, 0:64],
                                     kt_t[0:64, sl], start=True, stop=True)
                    nc.vector.tensor_scalar_add(kp[0:64, sl],
                                                pk2[0:64, :], bk_s[0:64])
                    pq1 = ps_misc.tile([128, 512], f32, tag="misc",
                                       name=f"pq1{p}_{qb}")
                    nc.tensor.matmul(pq1, wq_s[64:128, :],
                                     qt_t[64:128, sl], start=True, stop=True)
                    nc.vector.tensor_scalar_add(qp[64:128, sl],
                                                pq1[64:128, :], bq_s[64:128])
                    pq2 = ps_misc.tile([128, 512], f32, tag="misc",
                                       name=f"pq2{p}_{qb}")
                    nc.tensor.matmul(pq2[0:64, :], wq_s[0:64, 0:64],
                                     qt_t[0:64, sl], start=True, stop=True)
                    nc.vector.tensor_scalar_add(qp[0:64, sl],
                                                pq2[0:64, :], bq_s[0:64])
                qp_s.append(qp)
                kp_s.append(kp)

            emit_proj(0)
            for t in range(S // 128):
                v1t = vpool.tile([128, 65 * HPC], f32r, tag=f"v1_{t}",
                                 name=f"v1_{t}")
                nc.sync.dma_start(out=v1t, in_=v1_d[128 * t:128 * t + 128, :])
                v1_s.append(v1t)
            for fn in deferred_qt:
                fn()
            emit_proj(1)
            for pr in range(2):
                wfj = consts.tile([2 * HD, D], f32r, tag=f"wf{pr}",
                                  name=f"wf{pr}")
                nc.sync.dma_start(out=wfj,
                                  in_=wf_d[128 * pr:128 * pr + 128, :])
                wf_s.append(wfj)

            # ---------------- attention + interleaved fc ----------------
            oT_s = []
            for pr in range(2):
                oT = otp.tile([2 * HD, S], f32r, tag=f"oT{pr}", name=f"oT{pr}")
                oT_s.append(oT)

            NQB = S // 512           # outer q windows (512 wide)
            NKT = S // 128           # k tiles

            def emit_norm(p, qb, oc_t):
                # oc_t: sbuf [65, 1024] copy of the av accumulators for the
                # head pair (head 2p cols 0:512, head 2p+1 cols 512:1024;
                # row 64 = denominators). Normalize into the oT pair tile;
                # head B additionally goes through an identity-shift matmul
                # to land on partitions 64-127 (so fc can contract K=128).
                rsb = small.tile([65, 1024], f32r, tag="r",
                                 name=f"r{p}_{qb}")
                nc.vector.reciprocal(out=rsb[64:65, :], in_=oc_t[64:65, :])
                q0 = 512 * qb
                # head A (2p): normalize straight into rows 0-63
                rbpa = ps_misc.tile([HD, 512], f32, tag="rbp",
                                    name=f"rbpa{p}_{qb}")
                nc.tensor.matmul(rbpa, ones_s[64:65, :], rsb[64:65, 0:512],
                                 start=True, stop=True)
                nc.vector.tensor_mul(oT_s[p][0:64, q0:q0 + 512],
                                     rbpa, oc_t[0:64, 0:512])
                # head B (2p+1): normalize into a temp, shift to rows 64-127
                rbpb = ps_misc.tile([HD, 512], f32, tag="rbp",
                                    name=f"rbpb{p}_{qb}")
                nc.tensor.matmul(rbpb, ones_s[64:65, :], rsb[64:65, 512:1024],
                                 start=True, stop=True)
                oTb = small.tile([HD, 512], f32r, tag="oTb",
                                 name=f"oTb{p}_{qb}")
                nc.vector.tensor_mul(oTb, rbpb, oc_t[0:64, 512:1024])
                shp = ps_misc.tile([128, 512], f32, tag="rbp",
                                   name=f"shp{p}_{qb}")
                nc.tensor.matmul(shp, ish_s, oTb, start=True, stop=True)
                nc.vector.tensor_copy(oT_s[p][64:128, q0:q0 + 512],
                                      shp[64:128, :])

            # fc is emitted as single-matmul closures popped one per k-tile
            # iteration, so they never displace more than ~213ns of the
            # score->exp->av pipeline at a time.
            fc_state = {}

            def emit_fc_op(st, cb, pr, pool, tag):
                if cb == 0 and pr == 0:
                    fc_state["y_sb"] = ysb.tile([128, D], f32, tag="y",
                                                name=f"y{st}")
                if pr == 0:
                    fc_state["yp"] = pool.tile([128, 512], f32, tag=tag,
                                               name=f"yp{st}_{cb}")
                yp = fc_state["yp"]
                nc.tensor.matmul(
                    yp,
                    oT_s[pr][:, 128 * st:128 * st + 128],
                    wf_s[pr][:, 512 * cb:512 * cb + 512],
                    start=(pr == 0), stop=(pr == 1))
                if pr == 1:
                    y_sb = fc_state["y_sb"]
                    nc.vector.tensor_copy(y_sb[:, 512 * cb:512 * cb + 512], yp)
                    if cb == D // 512 - 1:
                        nc.sync.dma_start(
                            out=y_d[128 * st:128 * st + 128, :], in_=y_sb)

            # last q-window: pair-0 partials staged to SBUF during the final
            # attention round, pair-1 matmul + combine in the drain
            ya_st = {}

            def emit_fcA_op(st, cb):
                ypa = ps_misc.tile([128, 512], f32, tag="misc",
                                   name=f"ypa{st}_{cb}")
                nc.tensor.matmul(ypa,
                                 oT_s[0][:, 128 * st:128 * st + 128],
                                 wf_s[0][:, 512 * cb:512 * cb + 512],
                                 start=True, stop=True)
                ya = ysb.tile([128, 512], f32, tag="ya", bufs=8,
                              name=f"ya{st}_{cb}")
                nc.vector.tensor_copy(ya, ypa)
                ya_st[(st, cb)] = ya

            def emit_fcB_op(st, cb):
                if cb == 0:
                    fc_state["y_sb"] = ysb.tile([128, D], f32, tag="y",
                                                name=f"y{st}")
                # rotate three psum slots (2x score + the idle misc bank) so
                # the drain-phase matmul/copy chains pipeline deeper
                if (2 * st + cb) % 3 == 2:
                    ypb = ps_misc.tile([128, 512], f32, tag="misc",
                                       name=f"ypb{st}_{cb}")
                else:
                    ypb = ps_score.tile([128, 512], f32, tag="score",
                                        name=f"ypb{st}_{cb}")
                nc.tensor.matmul(ypb,
                                 oT_s[1][:, 128 * st:128 * st + 128],
                                 wf_s[1][:, 512 * cb:512 * cb + 512],
                                 start=True, stop=True)
                y_sb = fc_state["y_sb"]
                nc.vector.tensor_add(y_sb[:, 512 * cb:512 * cb + 512],
                                     ya_st[(st, cb)], ypb)
                if cb == D // 512 - 1:
                    nc.sync.dma_start(
                        out=y_d[128 * st:128 * st + 128, :], in_=y_sb)

            pending_norm = None
            fc_queue = []
            for qb in range(NQB):
                for p in range(2):
                    o_ps = []
                    for half in range(2):
                        o = ps_av.tile([65, 512], f32, tag="av",
                                       name=f"o{p}_{qb}_{half}")
                        o_ps.append(o)
                    q0 = 512 * qb
                    qa = qp_s[p][0:64, q0:q0 + 512]
                    qb_ = qp_s[p][64:128, q0:q0 + 512]
                    for kt in range(NKT):
                        ks = slice(128 * kt, 128 * kt + 128)
                        sc = ps_score.tile([128, 1024], f32, tag="score",
                                           name=f"sc{p}_{qb}_{kt}")
                        nc.tensor.matmul(sc[:, 0:512], kp_s[p][0:64, ks], qa,
                                         start=True, stop=True)
                        nc.tensor.matmul(sc[:, 512:1024],
                                         kp_s[p][64:128, ks], qb_,
                                         start=True, stop=True)
                        ex = expp.tile([128, 1024], f32r, tag="exp",
                                       name=f"ex{p}_{qb}_{kt}")
                        nc.scalar.activation(out=ex, in_=sc, func=EXP)
                        va = v1_s[kt][:, 65 * 2 * p:65 * 2 * p + 65]
                        vb = v1_s[kt][:, 65 * (2 * p + 1):65 * (2 * p + 1) + 65]
                        nc.tensor.matmul(o_ps[0], va, ex[:, 0:512],
                                         start=(kt == 0), stop=(kt == NKT - 1))
                        nc.tensor.matmul(o_ps[1], vb, ex[:, 512:1024],
                                         start=(kt == 0), stop=(kt == NKT - 1))
                        if kt == 2 and pending_norm is not None:
                            emit_norm(*pending_norm)
                            pending_norm = None
                            if qb == NQB - 1 and p == 1:
                                # pair-0 of the last window is normalized now;
                                # its fc partials can overlap this last round
                                for st_ in range(4 * qb, 4 * qb + 4):
                                    for cb_ in range(D // 512):
                                        fc_queue.append(
                                            lambda st=st_, cb=cb_:
                                                emit_fcA_op(st, cb))
                        if fc_queue:
                            fc_queue.pop(0)()
                    # evacuate the accumulators to SBUF quickly so the av
                    # psum slots free up; normalization is deferred
                    oc_t = small.tile([65, 1024], f32, tag="oc", bufs=4,
                                      name=f"oc{p}_{qb}")
                    nc.vector.tensor_copy(oc_t[:, 0:512], o_ps[0])
                    nc.vector.tensor_copy(oc_t[:, 512:1024], o_ps[1])
                    pending_norm = (p, qb, oc_t)
                # fc for this q-window needs both pairs' norms done
                emit_norm(*pending_norm)
                pending_norm = None
                # earlier windows' fc pops during later attention (misc psum
                # slot); the last window is split: pair-0 partials pop during
                # the final round, pair-1 + combine drain at the end
                if qb < NQB - 1:
                    for st in range(4 * qb, 4 * qb + 4):
                        for cb in range(D // 512):
                            for pr in range(2):
                                fc_queue.append(
                                    lambda st=st, cb=cb, pr=pr:
                                        emit_fc_op(st, cb, pr, ps_misc,
                                                   "misc"))
                # (last window's fcA ops are enqueued mid-round, above)
            while fc_queue:
                fc_queue.pop(0)()
            for st in range(S // 128 - 4, S // 128):
                for cb in range(D // 512):
                    emit_fcB_op(st, cb)

    nc.compile()
    return nc


def _prep(query, key, value, Wq, bq, Wk, bk, Wv, bv, Wfc, bfc):
    """Host-side sharding / layout prep. Returns (in_maps, bfc_eff)."""
    query = np.asarray(query, dtype=np.float32)
    key = np.asarray(key, dtype=np.float32)
    value = np.asarray(value, dtype=np.float32)
    Wq = np.asarray(Wq, np.float32); bq = np.asarray(bq, np.float32)
    Wk = np.asarray(Wk, np.float32); bk = np.asarray(bk, np.float32)
    Wv = np.asarray(Wv, np.float32); bv = np.asarray(bv, np.float32)
    Wfc = np.asarray(Wfc, np.float32); bfc = np.asarray(bfc, np.float32)

    scale = np.float32(1.0 / np.sqrt(HD))
    wq_t = np.ascontiguousarray(Wq.T) * scale        # [d, e], scale folded
    bq_sc = (bq * scale).reshape(HD, 1).astype(np.float32)
    wk_t = np.ascontiguousarray(Wk.T)
    bk_c = bk.reshape(HD, 1).astype(np.float32)
    # block-diagonal for head-pair packing: head A reads [0:64, 0:64],
    # head B reads rows 64:128 as [zeros | w] (row-tiled M=128 matmul)
    z = np.zeros((HD, HD), np.float32)
    wq_t2 = np.ascontiguousarray(np.block([[wq_t, z], [z, wq_t]]))
    wk_t2 = np.ascontiguousarray(np.block([[wk_t, z], [z, wk_t]]))
    bq_2 = np.ascontiguousarray(np.vstack([bq_sc, bq_sc]))
    bk_2 = np.ascontiguousarray(np.vstack([bk_c, bk_c]))

    # fold Wv / bv into fc
    A = np.empty((D, D), np.float32)
    bfc_eff = bfc.astype(np.float32).copy()
    for h in range(HEAD):
        Wfc_h = Wfc[:, HD * h:HD * h + HD]
        A[:, HD * h:HD * h + HD] = Wfc_h @ Wv
        bfc_eff += Wfc_h @ bv
    At = np.ascontiguousarray(A.T)                    # [ch, c]

    ishift = np.zeros((HD, 2 * HD), np.float32)
    ishift[np.arange(HD), HD + np.arange(HD)] = 1.0

    qT = np.ascontiguousarray(query.transpose(0, 2, 1))   # [B, D, S]
    kT = np.ascontiguousarray(key.transpose(0, 2, 1))

    in_maps = []
    for core in range(N_CORES):
        b, hg = core // 4, core % 4
        ch0 = CH * hg
        v1 = np.empty((S, 65 * HPC), np.float32)
        for j in range(HPC):
            v1[:, 65 * j:65 * j + 64] = value[b][:, ch0 + HD * j:ch0 + HD * j + HD]
            v1[:, 65 * j + 64] = 1.0
        in_maps.append({
            "qt": np.ascontiguousarray(qT[b][ch0:ch0 + CH]),
            "kt": np.ascontiguousarray(kT[b][ch0:ch0 + CH]),
            "v1": v1,
            "wqt": wq_t2,
            "wkt": wk_t2,
            "bq": bq_2,
            "bk": bk_2,
            "wfct": np.ascontiguousarray(At[ch0:ch0 + CH]),
            "ones": np.ones((1, HD), np.float32),
            "ishift": ishift,
        })
    return in_maps, bfc_eff


def _run_once(inputs):
    global LAST_RESULTS
    from concourse.bass_utils import run_bass_kernel_spmd

    if "nc" not in _CACHE:
        _CACHE["nc"] = _build()
    nc = _CACHE["nc"]

    in_maps, bfc_eff = _prep(**inputs)
    res = run_bass_kernel_spmd(nc, in_maps, core_ids=list(range(N_CORES)))
    LAST_RESULTS = res

    out = np.empty((B, S, D), np.float32)
    for b in range(B):
        acc = res.results[4 * b]["y"].astype(np.float32).copy()
        for hg in range(1, 4):
            acc += res.results[4 * b + hg]["y"]
        out[b] = acc + bfc_eff
    return out


def kernel(**inputs) -> np.ndarray:
    last_exc = None
    for attempt in range(3):
        try:
            out = _run_once(inputs)
            amax = float(np.abs(out).max())
            if np.isfinite(out).all() and 1e-6 < amax < 1e3:
                return out
            raise RuntimeError(f"implausible kernel output (absmax={amax})")
        except Exception as e:  # noqa: BLE001 - retry transient HW failures
            last_exc = e
            _CACHE.pop("nc", None)
            _CACHE["nonce"] = attempt + 1
    raise last_exc



# revision 14
# speedup vs baseline: 1.1924x; 1.1924x over previous
"""Multi-head attention (B=2, S=2048, D=1024, H=16) on 8 Trainium2 NeuronCores.

Sharding: core i handles batch b = i//4 and head-group hg = i%4 (4 heads).
The fc layer is sharded over its contraction dim (each core emits a partial
y summed on the host); Wv/bv are folded into Wfc/bfc on the host (exact).

Algorithm (v3):
  - Wq is folded into the K side on the host: score = q . k~ with
    k~ = (Wq^T Wk / sqrt(hd)) k + bias-row, so no on-device projections.
    log2(e) and a power-of-two fp8 range scale are folded in as well, so
    the device computes t = score*log2e and exponentiates as 2^t.
  - scores: fp8e4m3 DoubleRow matmuls (contraction 2x33 packs the 64 head
    channels + bias row), out [128 keys, 512 q] psum f32 at 0.5 cycles/row.
  - exp: split across three engines per k-tile: ScalarE exact exp->bf16
    (scale=ln2/ascale), and DVE/Pool Schraudolph (one tensor_scalar
    mult+add writing int16 exponent-bits through a bf16-tile bitcast,
    ~3% rel err on a fraction of tiles; end-to-end rel err ~1.3e-2).
  - AV transposed: exp tile is the stationary operand [128 keys, 128 q],
    V (bf16, with a ones column for the denominator) streams as moving
    [128, 65] -> out [q, 64ch|den] psum, 65 cycles per k-tile: softmax
    normalization becomes a per-partition reciprocal+scalar-multiply.
  - oTn [128 q, 64A|64B] bf16 pairs are PE-transposed (identity moving)
    to [128 ch, 128 q] and the fc runs K=256 over two bf16 matmuls per
    512-wide output tile; y is DMA'd to DRAM straight from PSUM.
"""


import sys

import numpy as np

if "/opt/trn_rl_repo" not in sys.path:
    sys.path.insert(0, "/opt/trn_rl_repo")

HEAD = 16
B, S, D = 2, 2048, 1024
HD = 64
HPC = 4          # heads per core
CH = HPC * HD    # channels per core
N_CORES = 8
NKT = S // 128   # k tiles
NQB = S // 512   # q windows
LOG2E = 1.4426950408889634
ASCALE = 2.0     # fp8 range scale folded into k~; undone in the exp scale

_CACHE = {}
LAST_RESULTS = None


def _exp_engine_seq():
    """Deterministic greedy assignment of the 128 exp ops to engines,
    balancing modeled per-op cost plus each engine's other workload.
    GPSIMD can't touch PSUM on TRN2, so only ScalarE and DVE qualify."""
    cost = {"A": 1038.0, "D": 1193.0}
    load = {"A": 4900.0, "D": 25500.0}
    seq = []
    for _ in range(NQB * 2 * NKT):
        e = min(cost, key=lambda k: load[k] + cost[k])
        load[e] += cost[e]
        seq.append(e)
    return seq


def _build():
    import concourse.tile as tile
    from concourse import bacc, mybir

    f32 = mybir.dt.float32
    bf16 = mybir.dt.bfloat16
    fp8 = mybir.dt.float8e4
    i16 = mybir.dt.int16
    EXP = mybir.ActivationFunctionType.Exp
    DR = mybir.MatmulPerfMode.DoubleRow
    MULT = mybir.AluOpType.mult
    ADD = mybir.AluOpType.add

    nc = bacc.Bacc("TRN2", target_bir_lowering=False, debug=False,
                   num_devices=N_CORES)

    # unused internal tensor whose name varies per retry: changes the BIR
    # content hash so a retry never reuses a possibly-corrupt cached NEFF
    nonce = _CACHE.get("nonce", 0)
    if nonce:
        nc.dram_tensor(f"retry_nonce_{nonce}", [1, 1], mybir.dt.float32)

    qt_d = nc.dram_tensor("qtil", [33, 2 * HPC, S], fp8, kind="ExternalInput")
    kt_d = nc.dram_tensor("ktil", [33, 2 * HPC, S], fp8, kind="ExternalInput")
    v1_d = nc.dram_tensor("v1", [128, NKT, HPC, 65], bf16,
                          kind="ExternalInput")
    wf_d = nc.dram_tensor("wf", [128, 2, D], bf16, kind="ExternalInput")
    id_d = nc.dram_tensor("iden", [128, 128], bf16, kind="ExternalInput")
    y_d = nc.dram_tensor("y", [S, D], f32, kind="ExternalOutput")

    eng_seq = _exp_engine_seq()
    act_scale = float(np.log(2.0) / ASCALE)
    sch_mul = float(128.0 / ASCALE)
    sch_add = float(127 * 128 - 5.5)

    with tile.TileContext(nc) as tc, nc.allow_low_precision(
            reason="bf16/fp8 operands feed f32-psum matmuls"):
        with (
            tc.tile_pool(name="inp", bufs=1) as inp,
            tc.tile_pool(name="expp", bufs=3) as expp,
            tc.tile_pool(name="otnp", bufs=5) as otnp,
            tc.tile_pool(name="otp", bufs=1) as otp,
            tc.tile_pool(name="recp", bufs=3) as recp,
            tc.tile_pool(name="ysbp", bufs=3) as ysbp,
            tc.tile_pool(name="ps_sc", bufs=2, space="PSUM") as ps_sc,
            tc.tile_pool(name="ps_av", bufs=2, space="PSUM") as ps_av,
            tc.tile_pool(name="ps_tp", bufs=1, space="PSUM") as ps_tp,
            tc.tile_pool(name="ps_y", bufs=1, space="PSUM") as ps_y,
        ):
            # ------------- inputs (chunked; critical path first) ----------
            qtil = inp.tile([33, 2 * HPC, S], fp8, tag="qtil")
            ktil = inp.tile([33, 2 * HPC, S], fp8, tag="ktil")
            v1 = inp.tile([128, NKT, HPC, 65], bf16, tag="v1")
            wf = inp.tile([128, 2, D], bf16, tag="wf")
            iden = inp.tile([128, 128], bf16, tag="iden")

            # first q-window of pair-0 heads gates the first matmul
            nc.sync.dma_start(out=ktil[:, 0:4, :], in_=kt_d[:, 0:4, :])
            nc.sync.dma_start(out=qtil[:, 0:4, 0:512], in_=qt_d[:, 0:4, 0:512])
            nc.sync.dma_start(out=iden, in_=id_d[:, :])
            for c in range(4):
                nc.sync.dma_start(out=v1[:, 4 * c:4 * c + 4, :, :],
                                  in_=v1_d[:, 4 * c:4 * c + 4, :, :])
            nc.sync.dma_start(out=ktil[:, 4:8, :], in_=kt_d[:, 4:8, :])
            nc.sync.dma_start(out=qtil[:, 4:8, 0:512], in_=qt_d[:, 4:8, 0:512])
            nc.sync.dma_start(out=qtil[:, :, 512:2048],
                              in_=qt_d[:, :, 512:2048])
            nc.sync.dma_start(out=wf, in_=wf_d[:, :, :])

            oT = []
            for p in range(2):
                t = otp.tile([128, S], bf16, tag=f"oT{p}", name=f"oT{p}")
                oT.append(t)

            ei = 0
            for qb in range(NQB):
                q0 = 512 * qb
                for p in range(2):
                    hA, hB = 2 * p, 2 * p + 1
                    # one 2KB bank per accumulator; qs blocks padded to 128
                    # floats so only the qs==0 matmul carries start=True (a
                    # start marks the whole 2KB zero-region, so interleaved
                    # groups must share a single start per bank)
                    avA = ps_av.tile([128, 512], f32, tag="av",
                                     name=f"avA{qb}_{p}")
                    avB = ps_av.tile([128, 512], f32, tag="av",
                                     name=f"avB{qb}_{p}")
                    # software-pipelined: scores/exp run 2 k-tiles ahead of
                    # the AV consumers so PE never waits on the exp engines
                    pend = []
                    for kt in range(NKT + 2):
                        if kt < NKT:
                            ks = slice(128 * kt, 128 * kt + 128)
                            sc = ps_sc.tile([128, 1024], f32, tag="sc",
                                            name=f"sc{qb}_{p}_{kt}")
                            nc.tensor.matmul(
                                sc[:, 0:512], ktil[:, 2 * hA:2 * hA + 2, ks],
                                qtil[:, 2 * hA:2 * hA + 2, q0:q0 + 512],
                                start=True, stop=True, perf_mode=DR)
                            nc.tensor.matmul(
                                sc[:, 512:1024], ktil[:, 2 * hB:2 * hB + 2, ks],
                                qtil[:, 2 * hB:2 * hB + 2, q0:q0 + 512],
                                start=True, stop=True, perf_mode=DR)
                            ex = expp.tile([128, 1024], bf16, tag="ex",
                                           name=f"ex{qb}_{p}_{kt}")
                            eng = eng_seq[ei]
                            ei += 1
                            if eng == "A":
                                nc.scalar.activation(out=ex, in_=sc, func=EXP,
                                                     scale=act_scale)
                            else:
                                e = nc.vector if eng == "D" else nc.gpsimd
                                e.tensor_scalar(
                                    out=ex.bitcast(i16), in0=sc,
                                    scalar1=sch_mul, scalar2=sch_add,
                                    op0=MULT, op1=ADD)
                            pend.append((kt, ex))
                        if kt >= 2:
                            akt, aex = pend[kt - 2]
                            st = akt == 0
                            sp = akt == NKT - 1
                            for h2, av, hh in ((0, avA, hA), (1, avB, hB)):
                                for qs in range(4):
                                    nc.tensor.matmul(
                                        av[:, 128 * qs:128 * qs + 65],
                                        aex[:, 512 * h2 + 128 * qs:
                                            512 * h2 + 128 * qs + 128],
                                        v1[:, akt, hh, :],
                                        start=st and qs == 0, stop=sp,
                                        skip_group_check=True)
                    # ---------------- normalize + transpose ----------------
                    otn = []
                    for qs in range(4):
                        t = otnp.tile([128, 128], bf16, tag="otn",
                                      name=f"otn{qb}_{p}_{qs}")
                        otn.append(t)
                    for h2, av in ((0, avA), (1, avB)):
                        rec = recp.tile([128, 4], f32, tag="rec",
                                        name=f"rec{qb}_{p}_{h2}")
                        nc.vector.reciprocal(out=rec,
                                             in_=av[:, 64:512:128])
                        for qs in range(4):
                            nc.vector.tensor_scalar(
                                out=otn[qs][:, 64 * h2:64 * h2 + 64],
                                in0=av[:, 128 * qs:128 * qs + 64],
                                scalar1=rec[:, qs:qs + 1], scalar2=None,
                                op0=MULT)
                    tp = ps_tp.tile([128, 512], bf16, tag="tp",
                                    name=f"tp{qb}_{p}")
                    for qs in range(4):
                        nc.tensor.matmul(tp[:, 128 * qs:128 * qs + 128],
                                         otn[qs], iden[:, :],
                                         is_transpose=True,
                                         start=qs == 0, stop=True,
                                         skip_group_check=True)
                    nc.scalar.copy(oT[p][:, q0:q0 + 512], tp)
                # ---------------- fc for this q-window ----------------
                for st4 in range(4):
                    stt = 4 * qb + st4
                    for cb in range(2):
                        yp = ps_y.tile([128, 512], f32, tag="y",
                                       name=f"y{stt}_{cb}")
                        nc.tensor.matmul(yp, oT[0][:, 128 * stt:128 * stt + 128],
                                         wf[:, 0, 512 * cb:512 * cb + 512],
                                         start=True, stop=False)
                        nc.tensor.matmul(yp, oT[1][:, 128 * stt:128 * stt + 128],
                                         wf[:, 1, 512 * cb:512 * cb + 512],
                                         start=False, stop=True)
                        ysb = ysbp.tile([128, 512], f32, tag="ysb",
                                        name=f"ysb{stt}_{cb}")
                        nc.vector.tensor_copy(ysb, yp)
                        nc.sync.dma_start(
                            out=y_d[128 * stt:128 * stt + 128,
                                    512 * cb:512 * cb + 512],
                            in_=ysb)

    nc.compile()
    return nc


def _prep(query, key, value, Wq, bq, Wk, bk, Wv, bv, Wfc, bfc):
    """Host-side sharding / layout prep. Returns (in_maps, bfc_eff)."""
    import ml_dtypes

    query = np.asarray(query, dtype=np.float32)
    key = np.asarray(key, dtype=np.float32)
    value = np.asarray(value, dtype=np.float32)
    Wq = np.asarray(Wq, np.float32); bq = np.asarray(bq, np.float32)
    Wk = np.asarray(Wk, np.float32); bk = np.asarray(bk, np.float32)
    Wv = np.asarray(Wv, np.float32); bv = np.asarray(bv, np.float32)
    Wfc = np.asarray(Wfc, np.float32); bfc = np.asarray(bfc, np.float32)

    s_hd = np.float32(1.0 / np.sqrt(HD))
    # fold Wq into the K side: score*log2e = q . (M k) + w . k   (per head)
    M = (Wq.T @ Wk) * (s_hd * LOG2E * ASCALE)          # [d, e]
    w_row = (bq @ Wk) * (s_hd * LOG2E * ASCALE)        # [e]

    # fold Wv / bv into fc
    A = np.empty((D, D), np.float32)
    bfc_eff = bfc.copy()
    for h in range(HEAD):
        Wfc_h = Wfc[:, HD * h:HD * h + HD]
        A[:, HD * h:HD * h + HD] = Wfc_h @ Wv
        bfc_eff += Wfc_h @ bv
    At = np.ascontiguousarray(A.T)                     # [ch, c]

    iden = np.eye(128, dtype=ml_dtypes.bfloat16)

    in_maps = []
    for core in range(N_CORES):
        b, hg = core // 4, core % 4
        ch0 = CH * hg
        qtil = np.zeros((33, 2 * HPC, S), np.float32)
        ktil = np.zeros((33, 2 * HPC, S), np.float32)
        v1 = np.empty((128, NKT, HPC, 65), np.float32)
        for h in range(HPC):
            qh = query[b][:, ch0 + HD * h:ch0 + HD * h + HD]   # [S, 64]
            kh = key[b][:, ch0 + HD * h:ch0 + HD * h + HD]
            kt = kh @ M.T                                      # [S, 64]
            qtil[0:32, 2 * h, :] = qh[:, 0:32].T
            qtil[0:32, 2 * h + 1, :] = qh[:, 32:64].T
            qtil[32, 2 * h, :] = 1.0
            ktil[0:32, 2 * h, :] = kt[:, 0:32].T
            ktil[0:32, 2 * h + 1, :] = kt[:, 32:64].T
            ktil[32, 2 * h, :] = kh @ w_row
            vh = value[b][:, ch0 + HD * h:ch0 + HD * h + HD]
            v1[:, :, h, 0:64] = vh.reshape(NKT, 128, HD).transpose(1, 0, 2)
            v1[:, :, h, 64] = 1.0
        wfc = np.empty((128, 2, D), np.float32)
        wfc[:, 0, :] = At[ch0:ch0 + 128]
        wfc[:, 1, :] = At[ch0 + 128:ch0 + 256]
        in_maps.append({
            "qtil": qtil.astype(ml_dtypes.float8_e4m3).view(np.uint8),
            "ktil": ktil.astype(ml_dtypes.float8_e4m3).view(np.uint8),
            "v1": v1.astype(ml_dtypes.bfloat16).view(np.uint16),
            "wf": wfc.astype(ml_dtypes.bfloat16).view(np.uint16),
            "iden": iden.view(np.uint16),
        })
    return in_maps, bfc_eff


def _run_once(inputs):
    global LAST_RESULTS
    from concourse.bass_utils import run_bass_kernel_spmd

    if "nc" not in _CACHE:
        _CACHE["nc"] = _build()
    nc = _CACHE["nc"]

    in_maps, bfc_eff = _prep(**inputs)
    res = run_bass_kernel_spmd(nc, in_maps, core_ids=list(range(N_CORES)))
    LAST_RESULTS = res

    out = np.empty((B, S, D), np.float32)
    for b in range(B):
        acc = res.results[4 * b]["y"].astype(np.float32).copy()
        for hg in range(1, 4):
            acc += res.results[4 * b + hg]["y"]
        out[b] = acc + bfc_eff
    return out


def kernel(**inputs) -> np.ndarray:
    last_exc = None
    for attempt in range(3):
        try:
            out = _run_once(inputs)
            amax = float(np.abs(out).max())
            if np.isfinite(out).all() and 1e-6 < amax < 1e3:
                return out
            raise RuntimeError(f"implausible kernel output (absmax={amax})")
        except Exception as e:  # noqa: BLE001 - retry transient HW failures
            last_exc = e
            _CACHE.pop("nc", None)
            _CACHE["nonce"] = attempt + 1
    raise last_exc
